# revision 11
# baseline (speedup 1.0000x reference)
"""Trainium2 Bass kernel for nn_AttentionBlock (ragged_sequence, 16 equal
segments of 2048 q/kv tokens, HID=256, QD=64) on 8 NeuronCores.

Sharding: 2 segments (4096 rows) per core, weights replicated, outputs
concatenated host-side (attention is block-diagonal per segment -> no
cross-core communication needed).
"""

import os
import sys

os.environ.setdefault("MYCRO_LOCAL_CACHE", "1")
if "/opt/trn_rl_repo" not in sys.path:
    sys.path.insert(0, "/opt/trn_rl_repo")

import numpy as np

HID = 256
QD = 64
LQ = 2048
LH = 2048
B = 16
NCORES = 8
SEGS = 2                  # segments per core
ROWS = SEGS * LQ          # 4096 q rows per core
EPS = 1e-5
SCALE = 1.0 / 8.0         # 1/sqrt(QD)

_built = {}               # (apply0,) -> (nc, names)


def _patch_act_tables():
    """Make the act-table pass choose the combined exp+ln table for every
    activation: blank all other tables (indices preserved so walrus's
    act_func_set_id remap stays correct). Avoids 100+ ACT_TABLE_LOADs
    (1.28us each) from alternating Exp/Ln table picks."""
    import functools
    import concourse.hw_specs as hw_specs
    import concourse.bacc as bacc_mod
    if getattr(hw_specs, "_attn_tables_patched", False):
        return
    orig = hw_specs.get_activation_tables

    @functools.cache
    def patched(arch):
        tabs = dict(orig(arch))
        joint = "natural_log_exp_and_others"
        assert joint in tabs, sorted(tabs)
        return {name: (funcs if name == joint else set())
                for name, funcs in tabs.items()}

    hw_specs.get_activation_tables = patched
    bacc_mod.get_activation_tables = patched
    hw_specs._attn_tables_patched = True


def _build(apply0: bool):
    """Build the per-core Bass graph. apply0: apply norm0 weight/bias on
    device (norm1 weight/bias is applied host-side when non-trivial)."""
    from concourse import bacc, bass, mybir, tile

    _patch_act_tables()

    dt = mybir.dt
    f32 = dt.float32
    f32r = dt.float32r
    bf16 = dt.bfloat16
    f8 = dt.float8e4
    AF = mybir.ActivationFunctionType
    Alu = mybir.AluOpType
    DR = mybir.MatmulPerfMode.DoubleRow

    nc = bacc.Bacc("TRN2", target_bir_lowering=False, debug=False,
                   enable_asserts=False)

    qT_d = nc.dram_tensor("qT", [HID, ROWS], bf16, kind="ExternalInput")
    q_d = nc.dram_tensor("q", [ROWS, HID], f32, kind="ExternalInput")
    hT_d = nc.dram_tensor("hT", [HID, ROWS], bf16, kind="ExternalInput")
    wqT_d = nc.dram_tensor("WQT", [HID, QD], bf16, kind="ExternalInput")
    wkT_d = nc.dram_tensor("WKT", [HID, QD], bf16, kind="ExternalInput")
    wvT_d = nc.dram_tensor("WVT", [HID, HID], bf16, kind="ExternalInput")
    fwT_d = nc.dram_tensor("FCWT", [HID, HID], bf16, kind="ExternalInput")
    fb_d = nc.dram_tensor("FCB", [1, HID], bf16, kind="ExternalInput")
    idt_d = nc.dram_tensor("IDT", [128, 128], bf16, kind="ExternalInput")
    if apply0:
        n0w_d = nc.dram_tensor("N0W", [128, HID], f32, kind="ExternalInput")
        n0b_d = nc.dram_tensor("N0B", [128, HID], f32, kind="ExternalInput")
    out_d = nc.dram_tensor("out", [ROWS, HID], f32, kind="ExternalOutput")

    qT_a, q_a, hT_a = qT_d.ap(), q_d.ap(), hT_d.ap()
    out_a = out_d.ap()

    NJT = LH // 128           # 16 j-tiles per segment
    NIT = LQ // 128           # 16 i-tiles per segment
    NIC = 2                   # 1024-col i-chunks per segment for scores
    ICW = LQ // NIC           # 1024

    with tile.TileContext(nc) as tc:
        with (
            tc.tile_pool(name="const", bufs=1) as cpool,
            tc.tile_pool(name="kqq", bufs=1) as kqq_pool,
            tc.tile_pool(name="vsb", bufs=1) as v_pool,
        ):
            # ---- constants ----
            wq_sb = cpool.tile([128, 2 * QD], bf16)     # [e, (chunk, c)]
            wk_sb = cpool.tile([128, 2 * QD], bf16)
            wv_sb = cpool.tile([128, 2 * HID], bf16)    # [e, (chunk, d)]
            fw_sb = cpool.tile([128, 2 * HID], bf16)   # fc_w.T chunks
            fb_sb = cpool.tile([1, HID], bf16)
            one_sb = cpool.tile([1, 128], bf16)
            idt_sb = cpool.tile([128, 128], bf16)
            for e in range(2):
                nc.sync.dma_start(wq_sb[:, e * QD:(e + 1) * QD],
                                  wqT_d.ap()[e * 128:(e + 1) * 128, :])
                nc.sync.dma_start(wk_sb[:, e * QD:(e + 1) * QD],
                                  wkT_d.ap()[e * 128:(e + 1) * 128, :])
                nc.sync.dma_start(wv_sb[:, e * HID:(e + 1) * HID],
                                  wvT_d.ap()[e * 128:(e + 1) * 128, :])
                nc.sync.dma_start(fw_sb[:, e * HID:(e + 1) * HID],
                                  fwT_d.ap()[e * 128:(e + 1) * 128, :])
            nc.sync.dma_start(fb_sb[:], fb_d.ap()[:, :])
            nc.sync.dma_start(idt_sb[:], idt_d.ap()[:, :])
            nc.vector.memset(one_sb[:], 1.0)
            eps_sb = cpool.tile([128, 1], f32)
            nc.vector.memset(eps_sb[:], EPS)
            nb3_sb = cpool.tile([128, 1], f32)
            nc.vector.memset(nb3_sb[:], -3.0)
            if apply0:
                n0w_sb = cpool.tile([128, HID], f32)
                n0b_sb = cpool.tile([128, HID], f32)
                nc.sync.dma_start(n0w_sb[:], n0w_d.ap()[:, :])
                nc.sync.dma_start(n0b_sb[:], n0b_d.ap()[:, :])

            # persistent activations
            kT_sb = kqq_pool.tile([64, ROWS], bf16)     # K^T  [c, j_global]
            qq_sb = kqq_pool.tile([64, ROWS], bf16)     # qq^T [c, i_global]
            # V with a ones column: per (seg, jt) a [128, 257] block (fp8)
            v_sb = v_pool.tile([128, SEGS * NJT * (HID + 1)], f8)

            # ---------------- phase 1: projections ----------------
            with (
                tc.tile_pool(name="qhT", bufs=1) as qh_pool,
                tc.tile_pool(name="pp_kq", bufs=2,
                             space=bass.MemorySpace.PSUM) as pp_kq,
                tc.tile_pool(name="pp_v", bufs=2,
                             space=bass.MemorySpace.PSUM) as pp_v,
            ):
                # load qT / hT as 8 tiles each of [128, 1024]
                qts, hts = [], []
                for e in range(2):
                    for c in range(ROWS // 1024):
                        t = qh_pool.tile([128, 1024], bf16, tag=f"qt{e}_{c}")
                        nc.sync.dma_start(
                            t[:], qT_a[e * 128:(e + 1) * 128,
                                       c * 1024:(c + 1) * 1024])
                        qts.append((e, c, t))
                        t2 = qh_pool.tile([128, 1024], bf16, tag=f"ht{e}_{c}")
                        nc.sync.dma_start(
                            t2[:], hT_a[e * 128:(e + 1) * 128,
                                        c * 1024:(c + 1) * 1024])
                        hts.append((e, c, t2))

                def _slice(tiles, e, col, width):
                    c, off = col // 1024, col % 1024
                    assert off + width <= 1024
                    for (te, tcid, t) in tiles:
                        if te == e and tcid == c:
                            return t[:, off:off + width]
                    raise KeyError

                # kT / qqT: out [64, 512] chunks accumulated over e
                for dst, w_sb, src in ((kT_sb, wk_sb, hts), (qq_sb, wq_sb, qts)):
                    for col in range(0, ROWS, 512):
                        ps = pp_kq.tile([64, 512], f32, tag="kq")
                        for e in range(2):
                            nc.tensor.matmul(
                                ps[:],
                                w_sb[:, e * QD:(e + 1) * QD],
                                _slice(src, e, col, 512),
                                start=(e == 0), stop=(e == 1))
                        nc.vector.tensor_copy(dst[:, col:col + 512], ps[:])

                # V row-layout with ones column
                for s in range(SEGS):
                    for jt in range(NJT):
                        ps = pp_v.tile([128, HID], f32, tag="v")
                        col = s * LH + jt * 128
                        for e in range(2):
                            nc.tensor.matmul(
                                ps[:],
                                _slice(hts, e, col, 128),
                                wv_sb[:, e * HID:(e + 1) * HID],
                                start=(e == 0), stop=(e == 1))
                        base = (s * NJT + jt) * (HID + 1)
                        nc.scalar.copy(v_sb[:, base:base + HID], ps[:])
                        nc.vector.memset(v_sb[:, base + HID:base + HID + 1],
                                         1.0)

            # ---------------- phase 2: attention + epilogue ----------------
            with (
                tc.tile_pool(name="pt", bufs=20) as pt_pool,
                tc.tile_pool(name="qrow", bufs=4) as q_pool,
                tc.tile_pool(name="ep", bufs=4) as ep_pool,
                tc.tile_pool(name="st8", bufs=8) as st8_pool,
                tc.tile_pool(name="outp", bufs=4) as o_pool,
                tc.tile_pool(name="ps_st", bufs=2,
                             space=bass.MemorySpace.PSUM) as ps_st,
                tc.tile_pool(name="ps_att", bufs=2,
                             space=bass.MemorySpace.PSUM) as ps_att,
                tc.tile_pool(name="ps_fc", bufs=1,
                             space=bass.MemorySpace.PSUM) as ps_fc,
                tc.tile_pool(name="ps_tp", bufs=1,
                             space=bass.MemorySpace.PSUM) as ps_tp,
            ):
                for s in range(SEGS):
                    for ic in range(NIC):
                        icol = s * LQ + ic * ICW
                        # scores^T + exp -> P^T pair tiles (fp8, DoubleRow)
                        pts = []
                        for jp in range(NJT // 2):
                            pt = pt_pool.tile([128, 2 * ICW], f8, tag="pt")
                            for half in range(2):
                                jt = 2 * jp + half
                                st = ps_st.tile([128, ICW], f32, tag="st")
                                for h in range(2):
                                    nc.tensor.matmul(
                                        st[:, h * 512:(h + 1) * 512],
                                        kT_sb[:, s * LH + jt * 128:
                                              s * LH + (jt + 1) * 128],
                                        qq_sb[:, icol + h * 512:
                                              icol + (h + 1) * 512],
                                        start=True, stop=True)
                                nc.scalar.activation(
                                    pt[:, half * ICW:(half + 1) * ICW],
                                    st[:], AF.Exp, scale=SCALE,
                                    bias=nb3_sb[:])
                            pts.append(pt)

                        for il in range(ICW // 128):   # i-tiles in this chunk
                            it = ic * (ICW // 128) + il
                            row0 = s * LQ + it * 128
                            att = ps_att.tile([128, HID + 1], f32, tag="att")
                            for jp in range(NJT // 2):
                                vb = (s * NJT + 2 * jp) * (HID + 1)
                                lhs3 = pts[jp][:].rearrange(
                                    "p (t i) -> p t i", t=2)[
                                        :, :, il * 128:(il + 1) * 128]
                                rhs3 = v_sb[:, vb:vb + 2 * (HID + 1)].rearrange(
                                    "p (t d) -> p t d", t=2)
                                nc.tensor.matmul(
                                    att[:], lhs3, rhs3,
                                    start=(jp == 0), stop=(jp == NJT // 2 - 1),
                                    perf_mode=DR)

                            # ---- epilogue for this i-tile ----
                            qt = q_pool.tile([128, HID], f32, tag="q")
                            nc.sync.dma_start(qt[:],
                                              q_a[row0:row0 + 128, :])
                            rden = st8_pool.tile([128, 1], f32, tag="rd")
                            nc.vector.reciprocal(rden[:],
                                                 att[:, HID:HID + 1])
                            x0 = ep_pool.tile([128, HID], f32, tag="x0")
                            nc.vector.scalar_tensor_tensor(
                                x0[:], att[:, 0:HID], rden[:].opt(), qt[:],
                                op0=Alu.mult, op1=Alu.add)
                            mv6 = st8_pool.tile([128, 6], f32, tag="mv6")
                            nc.vector.bn_stats(mv6[:], x0[:])
                            mv = st8_pool.tile([128, 2], f32, tag="mv")
                            nc.vector.bn_aggr(mv[:], mv6[:])
                            lnv = st8_pool.tile([128, 1], f32, tag="lnv")
                            nc.scalar.activation(lnv[:], mv[:, 1:2], AF.Ln,
                                                 bias=eps_sb[:])
                            rstd = st8_pool.tile([128, 1], f32, tag="rstd")
                            nc.scalar.activation(rstd[:], lnv[:], AF.Exp,
                                                 scale=-0.5)
                            z = ep_pool.tile([128, HID], bf16, tag="z")
                            nc.gpsimd.tensor_scalar(
                                z[:], x0[:], mv[:, 0:1].opt(), rstd[:].opt(),
                                op0=Alu.subtract, op1=Alu.mult)
                            if apply0:
                                z2 = ep_pool.tile([128, HID], bf16, tag="z2")
                                nc.gpsimd.tensor_tensor(z2[:], z[:], n0w_sb[:],
                                                        op=Alu.mult)
                                z3 = ep_pool.tile([128, HID], bf16, tag="z3")
                                nc.gpsimd.tensor_tensor(z3[:], z2[:], n0b_sb[:],
                                                        op=Alu.add)
                                zf = z3
                            else:
                                zf = z
                            # transpose zf (2x 128x128) then fc
                            hres = ps_fc.tile([128, HID], f32, tag="fc")
                            nc.tensor.matmul(hres[:], one_sb[:], fb_sb[:],
                                             start=True, stop=False)
                            for hh in range(2):
                                tp = ps_tp.tile([128, 128], bf16, tag="tp")
                                nc.tensor.transpose(
                                    tp[:], zf[:, hh * 128:(hh + 1) * 128],
                                    idt_sb[:])
                                zT = ep_pool.tile([128, 128], bf16,
                                                  tag=f"zT{hh}")
                                nc.vector.tensor_copy(zT[:], tp[:])
                                nc.tensor.matmul(
                                    hres[:], zT[:],
                                    fw_sb[:, hh * HID:(hh + 1) * HID],
                                    start=False, stop=(hh == 1))
                            y0 = ep_pool.tile([128, HID], f32, tag="y0")
                            nc.vector.scalar_tensor_tensor(
                                y0[:], hres[:], 0.0, zf[:],
                                op0=Alu.max, op1=Alu.add)
                            mv6b = st8_pool.tile([128, 6], f32, tag="mv6b")
                            nc.vector.bn_stats(mv6b[:], y0[:])
                            mvb = st8_pool.tile([128, 2], f32, tag="mvb")
                            nc.vector.bn_aggr(mvb[:], mv6b[:])
                            lnb = st8_pool.tile([128, 1], f32, tag="lnb")
                            nc.scalar.activation(lnb[:], mvb[:, 1:2], AF.Ln,
                                                 bias=eps_sb[:])
                            rstdb = st8_pool.tile([128, 1], f32, tag="rstdb")
                            nc.scalar.activation(rstdb[:], lnb[:], AF.Exp,
                                                 scale=-0.5)
                            ot = o_pool.tile([128, HID], f32, tag="ot")
                            nc.gpsimd.tensor_scalar(
                                ot[:], y0[:], mvb[:, 0:1].opt(),
                                rstdb[:].opt(),
                                op0=Alu.subtract, op1=Alu.mult)
                            nc.sync.dma_start(out_a[row0:row0 + 128, :],
                                              ot[:])

    nc.compile()
    return nc


def _get_nc(apply0: bool):
    key = (bool(apply0),)
    if key not in _built:
        _built[key] = _build(apply0)
    return _built[key]


def _shard(inputs, apply0):
    from concourse import mybir
    bf = mybir.dt.np(mybir.dt.bfloat16)

    q = np.ascontiguousarray(np.asarray(inputs["q"], dtype=np.float32))
    h = np.ascontiguousarray(np.asarray(inputs["h"], dtype=np.float32))
    WQ = np.asarray(inputs["WQ"], dtype=np.float32)
    WK = np.asarray(inputs["WK"], dtype=np.float32)
    WV = np.asarray(inputs["WV"], dtype=np.float32)
    fcw = np.asarray(inputs["fc_w"], dtype=np.float32)
    fcb = np.asarray(inputs["fc_b"], dtype=np.float32)

    WQT = np.ascontiguousarray(WQ.T).astype(bf)
    WKT = np.ascontiguousarray(WK.T).astype(bf)
    WVT = np.ascontiguousarray(WV.T).astype(bf)
    FCWT = np.ascontiguousarray(fcw.T).astype(bf)
    FCB = np.ascontiguousarray(fcb.reshape(1, HID)).astype(bf)
    IDT = np.eye(128, dtype=np.float32).astype(bf)

    in_maps = []
    for c in range(NCORES):
        sl = slice(c * ROWS, (c + 1) * ROWS)
        m = {
            "qT": np.ascontiguousarray(q[sl].T).astype(bf),
            "q": q[sl],
            "hT": np.ascontiguousarray(h[sl].T).astype(bf),
            "WQT": WQT, "WKT": WKT, "WVT": WVT,
            "FCWT": FCWT, "FCB": FCB, "IDT": IDT,
        }
        if apply0:
            m["N0W"] = np.ascontiguousarray(
                np.broadcast_to(np.asarray(inputs["norm0_w"], np.float32),
                                (128, HID)))
            m["N0B"] = np.ascontiguousarray(
                np.broadcast_to(np.asarray(inputs["norm0_b"], np.float32),
                                (128, HID)))
        in_maps.append(m)
    return in_maps


def _run(inputs, trace=False, tmpdir=None):
    from concourse import bass_utils

    n0w = np.asarray(inputs["norm0_w"], np.float32)
    n0b = np.asarray(inputs["norm0_b"], np.float32)
    n1w = np.asarray(inputs["norm1_w"], np.float32)
    n1b = np.asarray(inputs["norm1_b"], np.float32)
    apply0 = not (np.allclose(n0w, 1.0) and np.allclose(n0b, 0.0))
    apply1 = not (np.allclose(n1w, 1.0) and np.allclose(n1b, 0.0))

    nc = _get_nc(apply0)
    in_maps = _shard(inputs, apply0)
    res = bass_utils.run_bass_kernel_spmd(
        nc, in_maps, core_ids=list(range(NCORES)), trace=trace,
        tmpdir=tmpdir)
    out = np.concatenate([np.asarray(res.results[c]["out"])
                          for c in range(NCORES)], axis=0)
    if apply1:
        out = out * n1w[None, :] + n1b[None, :]
    return out.astype(np.float32), res


def kernel(**inputs):
    out, _ = _run(inputs, trace=False)
    return out


# revision 12
# speedup vs baseline: 1.5327x; 1.5327x over previous
"""Trainium2 Bass kernel for nn_AttentionBlock (ragged_sequence, 16 equal
segments of 2048 q/kv tokens, HID=256, QD=64) on 8 NeuronCores.

Sharding: 2 segments (4096 rows) per core, weights replicated, outputs
concatenated host-side (attention is block-diagonal per segment -> no
cross-core communication needed).
"""

import os
import sys

os.environ.setdefault("MYCRO_LOCAL_CACHE", "1")
if "/opt/trn_rl_repo" not in sys.path:
    sys.path.insert(0, "/opt/trn_rl_repo")

import numpy as np

HID = 256
QD = 64
LQ = 2048
LH = 2048
B = 16
NCORES = 8
SEGS = 2                  # segments per core
ROWS = SEGS * LQ          # 4096 q rows per core
EPS = 1e-5
SCALE = 1.0 / 8.0         # 1/sqrt(QD)

_built = {}               # (apply0,) -> (nc, names)


def _patch_act_tables():
    """Make the act-table pass choose the combined exp+ln table for every
    activation: blank all other tables (indices preserved so walrus's
    act_func_set_id remap stays correct). Avoids 100+ ACT_TABLE_LOADs
    (1.28us each) from alternating Exp/Ln table picks."""
    import functools
    import concourse.hw_specs as hw_specs
    import concourse.bacc as bacc_mod
    if getattr(hw_specs, "_attn_tables_patched", False):
        return
    orig = hw_specs.get_activation_tables

    @functools.cache
    def patched(arch):
        tabs = dict(orig(arch))
        joint = "natural_log_exp_and_others"
        assert joint in tabs, sorted(tabs)
        return {name: (funcs if name == joint else set())
                for name, funcs in tabs.items()}

    hw_specs.get_activation_tables = patched
    bacc_mod.get_activation_tables = patched
    hw_specs._attn_tables_patched = True


def _build(apply0: bool):
    """Build the per-core Bass graph. apply0: apply norm0 weight/bias on
    device (norm1 weight/bias is applied host-side when non-trivial)."""
    from concourse import bacc, bass, mybir, tile

    _patch_act_tables()

    dt = mybir.dt
    f32 = dt.float32
    f32r = dt.float32r
    bf16 = dt.bfloat16
    f8 = dt.float8e4
    AF = mybir.ActivationFunctionType
    Alu = mybir.AluOpType
    DR = mybir.MatmulPerfMode.DoubleRow

    nc = bacc.Bacc("TRN2", target_bir_lowering=False, debug=False,
                   enable_asserts=False)

    qT_d = nc.dram_tensor("qT", [HID, ROWS], bf16, kind="ExternalInput")
    q_d = nc.dram_tensor("q", [ROWS, HID], f32, kind="ExternalInput")
    hT_d = nc.dram_tensor("hT", [HID, ROWS], bf16, kind="ExternalInput")
    wqT_d = nc.dram_tensor("WQT", [HID, QD], bf16, kind="ExternalInput")
    wkT_d = nc.dram_tensor("WKT", [HID, QD], bf16, kind="ExternalInput")
    wvT_d = nc.dram_tensor("WVT", [HID, HID], bf16, kind="ExternalInput")
    fwT_d = nc.dram_tensor("FCWT", [HID, HID], bf16, kind="ExternalInput")
    fb_d = nc.dram_tensor("FCB", [1, HID], bf16, kind="ExternalInput")
    idt_d = nc.dram_tensor("IDT", [128, 128], bf16, kind="ExternalInput")
    if apply0:
        n0w_d = nc.dram_tensor("N0W", [128, HID], f32, kind="ExternalInput")
        n0b_d = nc.dram_tensor("N0B", [128, HID], f32, kind="ExternalInput")
    out_d = nc.dram_tensor("out", [ROWS, HID], f32, kind="ExternalOutput")

    qT_a, q_a, hT_a = qT_d.ap(), q_d.ap(), hT_d.ap()
    out_a = out_d.ap()

    NJT = LH // 128           # 16 j-tiles per segment
    NIT = LQ // 128           # 16 i-tiles per segment
    NIC = 2                   # 1024-col i-chunks per segment for scores
    ICW = LQ // NIC           # 1024

    with tile.TileContext(nc) as tc:
        with (
            tc.tile_pool(name="const", bufs=1) as cpool,
            tc.tile_pool(name="kqq", bufs=1) as kqq_pool,
            tc.tile_pool(name="vsb", bufs=1) as v_pool,
        ):
            # ---- constants ----
            wq_sb = cpool.tile([128, 2 * QD], bf16)     # [e, (chunk, c)]
            wk_sb = cpool.tile([128, 2 * QD], bf16)
            wv_sb = cpool.tile([128, 2 * HID], bf16)    # [e, (chunk, d)]
            fw_sb = cpool.tile([128, 2 * HID], bf16)   # fc_w.T chunks
            fb_sb = cpool.tile([1, HID], bf16)
            one_sb = cpool.tile([1, 128], bf16)
            idt_sb = cpool.tile([128, 128], bf16)
            for e in range(2):
                nc.sync.dma_start(wq_sb[:, e * QD:(e + 1) * QD],
                                  wqT_d.ap()[e * 128:(e + 1) * 128, :])
                nc.sync.dma_start(wk_sb[:, e * QD:(e + 1) * QD],
                                  wkT_d.ap()[e * 128:(e + 1) * 128, :])
                nc.sync.dma_start(wv_sb[:, e * HID:(e + 1) * HID],
                                  wvT_d.ap()[e * 128:(e + 1) * 128, :])
                nc.sync.dma_start(fw_sb[:, e * HID:(e + 1) * HID],
                                  fwT_d.ap()[e * 128:(e + 1) * 128, :])
            nc.sync.dma_start(fb_sb[:], fb_d.ap()[:, :])
            nc.sync.dma_start(idt_sb[:], idt_d.ap()[:, :])
            nc.vector.memset(one_sb[:], 1.0)
            eps_sb = cpool.tile([128, 1], f32)
            nc.vector.memset(eps_sb[:], EPS)
            nb3_sb = cpool.tile([128, 1], f32)
            nc.vector.memset(nb3_sb[:], -3.0)
            if apply0:
                n0w_sb = cpool.tile([128, HID], f32)
                n0b_sb = cpool.tile([128, HID], f32)
                nc.sync.dma_start(n0w_sb[:], n0w_d.ap()[:, :])
                nc.sync.dma_start(n0b_sb[:], n0b_d.ap()[:, :])

            # persistent activations
            kT_sb = kqq_pool.tile([64, ROWS], bf16)     # K^T  [c, j_global]
            qq_sb = kqq_pool.tile([64, ROWS], bf16)     # qq^T [c, i_global]
            # V with a ones column: per (seg, jt) a [128, 257] block (fp8)
            v_sb = v_pool.tile([128, SEGS * NJT * (HID + 1)], f8)

            # ---------------- phase 1: projections ----------------
            with (
                tc.tile_pool(name="qhT", bufs=1) as qh_pool,
                tc.tile_pool(name="pp_kq", bufs=4,
                             space=bass.MemorySpace.PSUM) as pp_kq,
                tc.tile_pool(name="pp_v", bufs=4,
                             space=bass.MemorySpace.PSUM) as pp_v,
            ):
                # load qT / hT as 8 tiles each of [128, 1024]
                qts, hts = [], []
                for e in range(2):
                    for c in range(ROWS // 1024):
                        t = qh_pool.tile([128, 1024], bf16, tag=f"qt{e}_{c}")
                        nc.sync.dma_start(
                            t[:], qT_a[e * 128:(e + 1) * 128,
                                       c * 1024:(c + 1) * 1024])
                        qts.append((e, c, t))
                        t2 = qh_pool.tile([128, 1024], bf16, tag=f"ht{e}_{c}")
                        nc.sync.dma_start(
                            t2[:], hT_a[e * 128:(e + 1) * 128,
                                        c * 1024:(c + 1) * 1024])
                        hts.append((e, c, t2))

                def _slice(tiles, e, col, width):
                    c, off = col // 1024, col % 1024
                    assert off + width <= 1024
                    for (te, tcid, t) in tiles:
                        if te == e and tcid == c:
                            return t[:, off:off + width]
                    raise KeyError

                # kT / qqT: out [64, 512] chunks accumulated over e
                for dst, w_sb, src in ((kT_sb, wk_sb, hts), (qq_sb, wq_sb, qts)):
                    for col in range(0, ROWS, 512):
                        ps = pp_kq.tile([64, 512], f32, tag="kq")
                        for e in range(2):
                            nc.tensor.matmul(
                                ps[:],
                                w_sb[:, e * QD:(e + 1) * QD],
                                _slice(src, e, col, 512),
                                start=(e == 0), stop=(e == 1))
                        nc.vector.tensor_copy(dst[:, col:col + 512], ps[:])

                # V row-layout with ones column
                for s in range(SEGS):
                    for jt in range(NJT):
                        ps = pp_v.tile([128, HID], f32, tag="v")
                        col = s * LH + jt * 128
                        for e in range(2):
                            nc.tensor.matmul(
                                ps[:],
                                _slice(hts, e, col, 128),
                                wv_sb[:, e * HID:(e + 1) * HID],
                                start=(e == 0), stop=(e == 1))
                        base = (s * NJT + jt) * (HID + 1)
                        nc.scalar.copy(v_sb[:, base:base + HID], ps[:])
                        nc.vector.memset(v_sb[:, base + HID:base + HID + 1],
                                         1.0)

            # ---------------- phase 2: attention + epilogue ----------------
            with (
                tc.tile_pool(name="pt", bufs=20) as pt_pool,
                tc.tile_pool(name="qrow", bufs=4) as q_pool,
                tc.tile_pool(name="ep", bufs=4) as ep_pool,
                tc.tile_pool(name="ep8", bufs=10) as ep8_pool,
                tc.tile_pool(name="st8", bufs=8) as st8_pool,
                tc.tile_pool(name="outp", bufs=4) as o_pool,
                tc.tile_pool(name="ps_st", bufs=2,
                             space=bass.MemorySpace.PSUM) as ps_st,
                tc.tile_pool(name="ps_att", bufs=2,
                             space=bass.MemorySpace.PSUM) as ps_att,
                tc.tile_pool(name="ps_fc", bufs=1,
                             space=bass.MemorySpace.PSUM) as ps_fc,
                tc.tile_pool(name="ps_tp", bufs=1,
                             space=bass.MemorySpace.PSUM) as ps_tp,
            ):
                for s in range(SEGS):
                    for ic in range(NIC):
                        icol = s * LQ + ic * ICW
                        # scores^T + exp -> P^T pair tiles (fp8, DoubleRow)
                        pts = []
                        for jp in range(NJT // 2):
                            pt = pt_pool.tile([128, 2 * ICW], f8, tag="pt")
                            for half in range(2):
                                jt = 2 * jp + half
                                st = ps_st.tile([128, ICW], f32, tag="st")
                                for h in range(2):
                                    nc.tensor.matmul(
                                        st[:, h * 512:(h + 1) * 512],
                                        kT_sb[:, s * LH + jt * 128:
                                              s * LH + (jt + 1) * 128],
                                        qq_sb[:, icol + h * 512:
                                              icol + (h + 1) * 512],
                                        start=True, stop=True)
                                nc.scalar.activation(
                                    pt[:, half * ICW:(half + 1) * ICW],
                                    st[:], AF.Exp, scale=SCALE,
                                    bias=nb3_sb[:])
                            pts.append(pt)

                        NIL = ICW // 128          # 8 i-tiles per chunk
                        mva0 = st8_pool.tile([128, 2 * NIL], f32, tag="mva0")
                        xs = []
                        for il in range(NIL):
                            att = ps_att.tile([128, HID + 1], f32, tag="att")
                            for jp in range(NJT // 2):
                                vb = (s * NJT + 2 * jp) * (HID + 1)
                                lhs3 = pts[jp][:].rearrange(
                                    "p (t i) -> p t i", t=2)[
                                        :, :, il * 128:(il + 1) * 128]
                                rhs3 = v_sb[:, vb:vb + 2 * (HID + 1)].rearrange(
                                    "p (t d) -> p t d", t=2)
                                nc.tensor.matmul(
                                    att[:], lhs3, rhs3,
                                    start=(jp == 0), stop=(jp == NJT // 2 - 1),
                                    perf_mode=DR)
                            it = ic * NIL + il
                            row0 = s * LQ + it * 128
                            qt = q_pool.tile([128, HID], f32, tag="q")
                            nc.sync.dma_start(qt[:], q_a[row0:row0 + 128, :])
                            rden = ep8_pool.tile([128, 1], f32, tag="rd")
                            nc.vector.reciprocal(rden[:], att[:, HID:HID + 1])
                            x0 = ep8_pool.tile([128, HID], f32, tag="x0")
                            nc.vector.scalar_tensor_tensor(
                                x0[:], att[:, 0:HID], rden[:].opt(), qt[:],
                                op0=Alu.mult, op1=Alu.add)
                            mv6 = st8_pool.tile([128, 6], f32, tag="mv6")
                            nc.vector.bn_stats(mv6[:], x0[:])
                            nc.vector.bn_aggr(mva0[:, 2 * il:2 * il + 2],
                                              mv6[:])
                            xs.append(x0)

                        ln8a = st8_pool.tile([128, NIL], f32, tag="ln8a")
                        nc.scalar.activation(
                            ln8a[:].rearrange("p (t o) -> p t o", o=1),
                            mva0[:].rearrange("p (t o) -> p t o", o=2)[:, :, 1:2],
                            AF.Ln, bias=eps_sb[:])
                        rstd8a = st8_pool.tile([128, NIL], f32, tag="r8a")
                        nc.scalar.activation(rstd8a[:], ln8a[:], AF.Exp,
                                             scale=-0.5)

                        mva1 = st8_pool.tile([128, 2 * NIL], f32, tag="mva1")
                        ys = []
                        for il in range(NIL):
                            x0 = xs[il]
                            z = ep8_pool.tile([128, HID], bf16, tag="z")
                            nc.vector.tensor_scalar(
                                z[:], x0[:], mva0[:, 2 * il:2 * il + 1].opt(),
                                rstd8a[:, il:il + 1].opt(),
                                op0=Alu.subtract, op1=Alu.mult)
                            if apply0:
                                z2 = ep_pool.tile([128, HID], bf16, tag="z2")
                                nc.gpsimd.tensor_tensor(z2[:], z[:], n0w_sb[:],
                                                        op=Alu.mult)
                                z3 = ep_pool.tile([128, HID], bf16, tag="z3")
                                nc.gpsimd.tensor_tensor(z3[:], z2[:], n0b_sb[:],
                                                        op=Alu.add)
                                zf = z3
                            else:
                                zf = z
                            hres = ps_fc.tile([128, HID], f32, tag="fc")
                            nc.tensor.matmul(hres[:], one_sb[:], fb_sb[:],
                                             start=True, stop=False)
                            for hh in range(2):
                                tp = ps_tp.tile([128, 128], bf16, tag="tp")
                                nc.tensor.transpose(
                                    tp[:], zf[:, hh * 128:(hh + 1) * 128],
                                    idt_sb[:])
                                zT = ep_pool.tile([128, 128], bf16,
                                                  tag=f"zT{hh}")
                                nc.vector.tensor_copy(zT[:], tp[:])
                                nc.tensor.matmul(
                                    hres[:], zT[:],
                                    fw_sb[:, hh * HID:(hh + 1) * HID],
                                    start=False, stop=(hh == 1))
                            y0 = ep8_pool.tile([128, HID], f32, tag="y0")
                            nc.vector.scalar_tensor_tensor(
                                y0[:], hres[:], 0.0, zf[:],
                                op0=Alu.max, op1=Alu.add)
                            mv6b = st8_pool.tile([128, 6], f32, tag="mv6b")
                            nc.vector.bn_stats(mv6b[:], y0[:])
                            nc.vector.bn_aggr(mva1[:, 2 * il:2 * il + 2],
                                              mv6b[:])
                            ys.append(y0)

                        ln8b = st8_pool.tile([128, NIL], f32, tag="ln8b")
                        nc.scalar.activation(
                            ln8b[:].rearrange("p (t o) -> p t o", o=1),
                            mva1[:].rearrange("p (t o) -> p t o", o=2)[:, :, 1:2],
                            AF.Ln, bias=eps_sb[:])
                        rstd8b = st8_pool.tile([128, NIL], f32, tag="r8b")
                        nc.scalar.activation(rstd8b[:], ln8b[:], AF.Exp,
                                             scale=-0.5)

                        for il in range(NIL):
                            it = ic * NIL + il
                            row0 = s * LQ + it * 128
                            b1 = st8_pool.tile([128, 1], f32, tag="b1")
                            nc.vector.tensor_scalar(
                                b1[:], mva1[:, 2 * il:2 * il + 1],
                                rstd8b[:, il:il + 1].opt(), -1.0,
                                op0=Alu.mult, op1=Alu.mult)
                            ot = o_pool.tile([128, HID], f32, tag="ot")
                            nc.scalar.activation(
                                ot[:], ys[il][:], AF.Identity,
                                bias=b1[:], scale=rstd8b[:, il:il + 1].opt())
                            nc.sync.dma_start(out_a[row0:row0 + 128, :],
                                              ot[:])

    nc.compile()
    return nc


def _get_nc(apply0: bool):
    key = (bool(apply0),)
    if key not in _built:
        _built[key] = _build(apply0)
    return _built[key]


def _shard(inputs, apply0):
    from concourse import mybir
    bf = mybir.dt.np(mybir.dt.bfloat16)

    q = np.ascontiguousarray(np.asarray(inputs["q"], dtype=np.float32))
    h = np.ascontiguousarray(np.asarray(inputs["h"], dtype=np.float32))
    WQ = np.asarray(inputs["WQ"], dtype=np.float32)
    WK = np.asarray(inputs["WK"], dtype=np.float32)
    WV = np.asarray(inputs["WV"], dtype=np.float32)
    fcw = np.asarray(inputs["fc_w"], dtype=np.float32)
    fcb = np.asarray(inputs["fc_b"], dtype=np.float32)

    WQT = np.ascontiguousarray(WQ.T).astype(bf)
    WKT = np.ascontiguousarray(WK.T).astype(bf)
    WVT = np.ascontiguousarray(WV.T).astype(bf)
    FCWT = np.ascontiguousarray(fcw.T).astype(bf)
    FCB = np.ascontiguousarray(fcb.reshape(1, HID)).astype(bf)
    IDT = np.eye(128, dtype=np.float32).astype(bf)

    in_maps = []
    for c in range(NCORES):
        sl = slice(c * ROWS, (c + 1) * ROWS)
        m = {
            "qT": np.ascontiguousarray(q[sl].T).astype(bf),
            "q": q[sl],
            "hT": np.ascontiguousarray(h[sl].T).astype(bf),
            "WQT": WQT, "WKT": WKT, "WVT": WVT,
            "FCWT": FCWT, "FCB": FCB, "IDT": IDT,
        }
        if apply0:
            m["N0W"] = np.ascontiguousarray(
                np.broadcast_to(np.asarray(inputs["norm0_w"], np.float32),
                                (128, HID)))
            m["N0B"] = np.ascontiguousarray(
                np.broadcast_to(np.asarray(inputs["norm0_b"], np.float32),
                                (128, HID)))
        in_maps.append(m)
    return in_maps


def _run(inputs, trace=False, tmpdir=None):
    from concourse import bass_utils

    n0w = np.asarray(inputs["norm0_w"], np.float32)
    n0b = np.asarray(inputs["norm0_b"], np.float32)
    n1w = np.asarray(inputs["norm1_w"], np.float32)
    n1b = np.asarray(inputs["norm1_b"], np.float32)
    apply0 = not (np.allclose(n0w, 1.0) and np.allclose(n0b, 0.0))
    apply1 = not (np.allclose(n1w, 1.0) and np.allclose(n1b, 0.0))

    nc = _get_nc(apply0)
    in_maps = _shard(inputs, apply0)
    res = bass_utils.run_bass_kernel_spmd(
        nc, in_maps, core_ids=list(range(NCORES)), trace=trace,
        tmpdir=tmpdir)
    out = np.concatenate([np.asarray(res.results[c]["out"])
                          for c in range(NCORES)], axis=0)
    if apply1:
        out = out * n1w[None, :] + n1b[None, :]
    return out.astype(np.float32), res


def kernel(**inputs):
    out, _ = _run(inputs, trace=False)
    return out
